# revision 16
# baseline (speedup 1.0000x reference)
"""Trainium2 Bass kernel for Points3DLoss (robust chamfer loss).

Computes, for inputs obs (2,16,4096,3) and pred (2,16,2048,3):
  d[bt,n]  = min_m |obs[bt,n] - pred[bt,m]|^2          (chamfer, per frame)
  res      = sqrt(d) reshaped to (B, T*N)
  med, mad = lower-median robust stats per batch row (on detached res)
  w        = bisquare weights; loss = 0.5 * sum(w * res^2)

Strategy (v3):
- Data-parallel over the 32 frames (4 per core).
- PE matmuls in bf16 split precision (hi/lo) with K=13 contraction rows
  computing z' = -0.5*|a-b|^2 directly in PSUM (no |a|^2-2ab cancellation).
  bf16 streams 1 col/cycle vs fp32's 4.
- Row max over the 2048 pred cols drained by three engines in parallel:
  DVE tensor_reduce direct from PSUM (X1 cols), ACT stages the rest to
  fp16 SBUF where GPSIMD max-reduces it (ports disjoint from DVE's).
- d gathered per batch row with subgroup AllGathers ({0-3},{4-7}), first
  half issued mid-loop to overlap the collective with compute.
- Tail (per core, on its own batch row): med/mad via value-space bisection
  counting on fp16 residuals, the count pass split DVE (is_lt cache-reduce)
  || ACT (Sign-activation accumulate); both partials are combined by two
  accumulating PE matmuls. Bisquare weighted sum, row loss DMA'd out; host
  adds core0 + core4 rows.
"""

import sys

if '/opt/trn_rl_repo' not in sys.path:
    sys.path.insert(0, '/opt/trn_rl_repo')

import numpy as np

B, T, N_OBS, M_PRED = 2, 16, 4096, 2048
BT = B * T
NCORES = 8
F = BT // NCORES          # frames per core = 4
CH = N_OBS // 128         # obs chunks per frame = 32
COLS = F * CH             # d columns per core = 128
NROW = T * N_OBS          # residuals per batch row = 65536
K_MED = 32768.0           # rank (1-based) of lower median
TUNE = 4.6851
MADSTD = 0.67449

X1 = 440                  # PSUM-direct max cols per chunk (DVE tensor_reduce)
X2 = M_PRED - X1          # ACT-staged fp16 cols: DVE tensor_tensor max fold
                          # (2 read ports) then cache-reduce of the half
N_ITERS = 11              # bisection iterations (bracket [0,2*T0])
TAIL_ACT = False          # split tail count passes DVE || ACT (Sign trick)
FD1 = 280                 # tail count cols on DVE; rest Sign-counted on ACT
FD2 = 512 - FD1
GROUPS = [[0, 1, 2, 3], [4, 5, 6, 7]]

_CACHE = {}


def _build_nc(stage="D"):
    import concourse.bacc as bacc
    import concourse.tile as tile
    from concourse import mybir
    from contextlib import ExitStack

    A = mybir.AluOpType
    AF = mybir.ActivationFunctionType
    f32 = mybir.dt.float32
    f16 = mybir.dt.float16
    bf16 = mybir.dt.bfloat16
    X = mybir.AxisListType.X

    nc = bacc.Bacc("TRN2", target_bir_lowering=False, debug=False,
                   num_devices=NCORES)

    obs_in = nc.dram_tensor("obs_in", [13, F * N_OBS], bf16,
                            kind="ExternalInput").ap()
    pred_in = nc.dram_tensor("pred_in", [13, F * M_PRED], bf16,
                             kind="ExternalInput").ap()
    out_d = nc.dram_tensor("out", [1, 1], f32, kind="ExternalOutput").ap()

    def emit(tc, pp, stack):
        OBSL = pp.tile([13, F * N_OBS], bf16, name="OBSL", tag="OBSL")
        PREDL = pp.tile([13, F * M_PRED], bf16, name="PREDL", tag="PREDL")
        for f in range(F):
            nc.sync.dma_start(out=PREDL[:, f * M_PRED:(f + 1) * M_PRED],
                              in_=pred_in[:, f * M_PRED:(f + 1) * M_PRED])
            nc.sync.dma_start(out=OBSL[:, f * N_OBS:(f + 1) * N_OBS],
                              in_=obs_in[:, f * N_OBS:(f + 1) * N_OBS])

        zP = pp.tile([128, COLS], f32, name="zP", tag="zP")
        zG = pp.tile([128, COLS], f32, name="zG", tag="zG")
        junkG = pp.tile([128, X2], f16, name="junkG", tag="junkG")
        g = pp.tile([128, 512], f32, name="g", tag="g")

        dp = stack.enter_context(tc.tile_pool(name="dram", bufs=1,
                                              space="DRAM"))
        cc_in = []
        cc_out = []
        for h in range(2):
            cc_in.append(dp.tile([128, 64], f32, name=f"cc_in{h}"))
            cc_out.append(dp.tile([4, 128, 64], f32, name=f"cc_out{h}"))

        def gather_half(h):
            dh = pp.tile([128, 64], f32, name=f"dh{h}", tag=f"dh{h}")
            lo = h * 64
            nc.vector.tensor_scalar(out=dh, in0=zG[:, lo:lo + 64],
                                    scalar1=-2.0, scalar2=0.0,
                                    op0=A.mult, op1=A.max)
            nc.sync.dma_start(out=cc_in[h], in_=dh)
            nc.gpsimd.collective_compute(
                "AllGather", A.bypass, replica_groups=GROUPS,
                ins=[cc_in[h][:]], outs=[cc_out[h][:]])
            nc.sync.dma_start(
                out=g[:, h * 256:(h + 1) * 256].rearrange(
                    "p (r c) -> p r c", r=4),
                in_=cc_out[h].rearrange("r p c -> p r c"))

        # --- main loop: z' = -0.5*|a-b|^2 via K=13 bf16 matmul ------------
        with tc.tile_pool(name="mm", bufs=2, space="PSUM") as mmp, \
             tc.tile_pool(name="stg", bufs=3) as stgp:
            for f in range(F):
                for c in range(CH):
                    col = f * CH + c
                    ps = mmp.tile([128, M_PRED], f32, name="mmps", tag="mmps")
                    lhsT = OBSL[:, f * N_OBS + c * 128:
                                f * N_OBS + (c + 1) * 128]
                    for q in range(4):
                        nc.tensor.matmul(
                            ps[:, q * 512:(q + 1) * 512], lhsT=lhsT,
                            rhs=PREDL[:, f * M_PRED + q * 512:
                                      f * M_PRED + (q + 1) * 512],
                            start=True, stop=True)
                    staged = stgp.tile([128, X2], f16, name="stg", tag="stg")
                    nc.scalar.copy(out=staged, in_=ps[:, X1:M_PRED])
                    nc.vector.tensor_reduce(
                        out=zP[:, col:col + 1], in_=ps[:, 0:X1], axis=X,
                        op=A.max)
                    half = X2 // 2
                    quart = half // 2
                    fold = stgp.tile([128, half], f16, name="fold",
                                     tag="fold")
                    nc.vector.tensor_tensor(
                        out=fold, in0=staged[:, 0:half],
                        in1=staged[:, half:2 * half], op=A.max)
                    fold2 = stgp.tile([128, quart], f16, name="fold2",
                                      tag="fold2")
                    nc.vector.tensor_tensor(
                        out=fold2, in0=fold[:, 0:quart],
                        in1=fold[:, quart:2 * quart], op=A.max)
                    nc.vector.tensor_scalar(
                        out=junkG[:, 0:quart], in0=fold2, scalar1=-1e30,
                        scalar2=zP[:, col:col + 1], op0=A.max, op1=A.max,
                        accum_out=zG[:, col:col + 1])
                if f == 1:
                    gather_half(0)
            gather_half(1)

        # --- tail: med/mad via value bisection on fp16 residuals ----------
        r16 = pp.tile([128, 512], f16, name="r16", tag="r16")
        nc.scalar.activation(out=r16, in_=g, func=AF.Sqrt)

        ones128 = pp.tile([128, 128], f32, name="ones128", tag="ones128")
        nc.vector.memset(ones128, 1.0)
        halfm = pp.tile([128, 128], f32, name="halfm", tag="halfm")
        nc.vector.memset(halfm, 0.5)
        half1 = pp.tile([128, 1], f32, name="half1", tag="half1")
        nc.vector.memset(half1, 0.5)

        cnt = pp.tile([128, 1], f32, name="cnt", tag="cnt")
        acc = pp.tile([128, 1], f32, name="acc", tag="acc")
        dT = pp.tile([128, 1], f32, name="dT", tag="dT")
        jk16 = junkG[:, 0:FD1]
        jkA = pp.tile([128, FD2], f16, name="jkA", tag="jkA")

        bp = stack.enter_context(tc.tile_pool(name="bis_ps", bufs=2,
                                              space="PSUM"))

        # count(vals < T) split: DVE is_lt on cols [0:FD1], ACT Sign on
        # [FD1:512] (sum of sign(T - x) = c_below - c_above); combined by
        # two accumulating matmuls: tot = sum(cnt) + 0.5*sum(acc), compared
        # against K - 64*FD2.
        K_ADJ = K_MED - 64.0 * FD2

        def bisect(vals, tag, T0):
            Tt = pp.tile([128, 1], f32, name=f"T_{tag}", tag=f"T_{tag}")
            nc.vector.memset(Tt, T0)
            for j in range(N_ITERS):
                step = float(T0 / 2 ** (j + 1))
                tot = bp.tile([128, 1], f32, name=f"tot_{tag}", tag="tot")
                if TAIL_ACT:
                    nc.vector.tensor_scalar(
                        out=jk16, in0=vals[:, 0:FD1], scalar1=Tt[:, 0:1],
                        scalar2=None, op0=A.is_lt, op1=A.add, accum_out=cnt)
                    nc.scalar.activation(
                        out=jkA, in_=vals[:, FD1:512], func=AF.Sign,
                        bias=Tt[:, 0:1], scale=-1.0, accum_out=acc)
                    nc.tensor.matmul(tot, lhsT=ones128, rhs=cnt,
                                     start=True, stop=False)
                    nc.tensor.matmul(tot, lhsT=halfm, rhs=acc,
                                     start=False, stop=True)
                    kcmp = K_ADJ
                else:
                    nc.vector.tensor_scalar(
                        out=junkG[:, 0:512], in0=vals, scalar1=Tt[:, 0:1],
                        scalar2=None, op0=A.is_lt, op1=A.add, accum_out=cnt)
                    nc.tensor.matmul(tot, lhsT=ones128, rhs=cnt,
                                     start=True, stop=True)
                    kcmp = K_MED
                nc.vector.tensor_scalar(
                    out=dT, in0=tot, scalar1=kcmp, scalar2=2.0 * step,
                    op0=A.is_lt, op1=A.mult)
                nc.vector.scalar_tensor_tensor(
                    out=Tt, in0=dT, scalar=step, op0=A.subtract, op1=A.add,
                    in1=Tt)
            return Tt

        med = bisect(r16, "med", 2.0)
        negmed = pp.tile([128, 1], f32, name="negmed", tag="negmed")
        nc.vector.tensor_scalar(out=negmed, in0=med, scalar1=-1.0,
                                scalar2=None, op0=A.mult)
        u16 = pp.tile([128, 512], f16, name="u16", tag="u16")
        nc.scalar.activation(out=u16, in_=r16, func=AF.Abs,
                             bias=negmed[:, 0:1], scale=1.0)
        mad = bisect(u16, "mad", 0.5)

        # --- loss = 0.5 * sum(w * d), w = relu(1 - d/(TUNE*std)^2)^2 ------
        c1 = pp.tile([128, 1], f32, name="c1", tag="c1")
        nc.vector.tensor_scalar(out=c1, in0=mad, scalar1=TUNE / MADSTD,
                                scalar2=None, op0=A.mult)
        cs2 = pp.tile([128, 1], f32, name="cs2", tag="cs2")
        nc.vector.tensor_tensor(out=cs2, in0=c1, in1=c1, op=A.mult)
        inv = pp.tile([128, 1], f32, name="inv", tag="inv")
        nc.vector.reciprocal(inv, cs2)

        t1 = pp.tile([128, 512], f32, name="t1", tag="t1")
        nc.vector.tensor_scalar(out=t1, in0=g, scalar1=inv[:, 0:1],
                                scalar2=None, op0=A.mult)
        v = pp.tile([128, 512], f32, name="v", tag="v")
        nc.scalar.activation(out=v, in_=t1, func=AF.Relu,
                             bias=1.0, scale=-1.0)
        y = pp.tile([128, 512], f32, name="y", tag="y")
        nc.vector.tensor_tensor(out=y, in0=v, in1=g, op=A.mult)
        S = pp.tile([128, 1], f32, name="S", tag="S")
        jkf = pp.tile([128, 512], f32, name="jkf", tag="jkf")
        nc.vector.scalar_tensor_tensor(
            out=jkf, in0=y, scalar=1.0, op0=A.bypass, op1=A.mult,
            in1=v, accum_out=S)

        ls = bp.tile([1, 1], f32, name="ls")
        nc.tensor.matmul(ls, lhsT=half1, rhs=S, start=True, stop=True)
        ls_sb = pp.tile([1, 1], f32, name="ls_sb", tag="ls_sb")
        nc.scalar.copy(out=ls_sb, in_=ls)
        nc.sync.dma_start(out=out_d, in_=ls_sb)

    from contextlib import ExitStack
    with tile.TileContext(nc) as tc, ExitStack() as stack:
        pp = stack.enter_context(tc.tile_pool(name="persist", bufs=1))
        emit(tc, pp, stack)

    nc.compile()
    return nc


def _split16(x64, dt):
    hi = x64.astype(dt)
    lo = (x64 - hi.astype(np.float64)).astype(dt)
    return hi, lo


def _shard_inputs(points3d_obs, points3d_pred):
    import ml_dtypes
    bf16 = ml_dtypes.bfloat16
    obs = np.asarray(points3d_obs, dtype=np.float32).reshape(BT, N_OBS, 3)
    pred = np.asarray(points3d_pred, dtype=np.float32).reshape(BT, M_PRED, 3)
    in_maps = []
    for core in range(NCORES):
        so = obs[core * F:(core + 1) * F]       # [F, N, 3]
        sp = pred[core * F:(core + 1) * F]      # [F, M, 3]

        ha, la = _split16(so.astype(np.float64), bf16)
        hna, lna = _split16(-0.5 * (so.astype(np.float64) ** 2).sum(-1), bf16)
        hb, lb = _split16(sp.astype(np.float64), bf16)
        hnb, lnb = _split16(-0.5 * (sp.astype(np.float64) ** 2).sum(-1), bf16)

        onesN = np.ones((F, N_OBS), bf16)
        onesM = np.ones((F, M_PRED), bf16)

        # [13, F*N]: hi/lo(-0.5|a|^2), ha, la, ha, 1, 1
        obs_rows = np.stack([
            hna, lna,
            ha[..., 0], ha[..., 1], ha[..., 2],
            la[..., 0], la[..., 1], la[..., 2],
            ha[..., 0], ha[..., 1], ha[..., 2],
            onesN, onesN,
        ], axis=0).reshape(13, F * N_OBS)
        # [13, F*M]: 1, 1, hb, hb, lb, hi/lo(-0.5|b|^2)
        pred_rows = np.stack([
            onesM, onesM,
            hb[..., 0], hb[..., 1], hb[..., 2],
            hb[..., 0], hb[..., 1], hb[..., 2],
            lb[..., 0], lb[..., 1], lb[..., 2],
            hnb, lnb,
        ], axis=0).reshape(13, F * M_PRED)

        in_maps.append({
            "obs_in": np.ascontiguousarray(obs_rows),
            "pred_in": np.ascontiguousarray(pred_rows),
        })
    return in_maps


def _get_nc(stage="D"):
    key = f"nc_{stage}"
    if key not in _CACHE:
        _CACHE[key] = _build_nc(stage)
    return _CACHE[key]


def run(points3d_obs, points3d_pred, stage="D", **kwargs):
    """Run on hardware; kwargs forwarded to run_bass_kernel_spmd."""
    from concourse.bass_utils import run_bass_kernel_spmd
    nc = _get_nc(stage)
    in_maps = _shard_inputs(points3d_obs, points3d_pred)
    res = run_bass_kernel_spmd(nc, in_maps, list(range(NCORES)), **kwargs)
    return res


def kernel(points3d_obs, points3d_pred):
    res = run(points3d_obs, points3d_pred)
    loss = (np.float32(res.results[0]["out"][0, 0])
            + np.float32(res.results[4]["out"][0, 0]))
    return np.asarray(loss, dtype=np.float32).reshape(())


# revision 17
# speedup vs baseline: 1.2350x; 1.2350x over previous
"""Trainium2 Bass kernel for Points3DLoss (robust chamfer loss).

Computes, for inputs obs (2,16,4096,3) and pred (2,16,2048,3):
  d[bt,n]  = min_m |obs[bt,n] - pred[bt,m]|^2          (chamfer, per frame)
  res      = sqrt(d) reshaped to (B, T*N)
  med, mad = lower-median robust stats per batch row (on detached res)
  w        = bisquare weights; loss = 0.5 * sum(w * res^2)

Strategy (v3):
- Data-parallel over the 32 frames (4 per core).
- PE matmuls in bf16 split precision (hi/lo) with K=13 contraction rows
  computing z' = -0.5*|a-b|^2 directly in PSUM (no |a|^2-2ab cancellation).
  bf16 streams 1 col/cycle vs fp32's 4.
- Row max over the 2048 pred cols drained by three engines in parallel:
  DVE tensor_reduce direct from PSUM (X1 cols), ACT stages the rest to
  fp16 SBUF where GPSIMD max-reduces it (ports disjoint from DVE's).
- d gathered per batch row with subgroup AllGathers ({0-3},{4-7}), first
  half issued mid-loop to overlap the collective with compute.
- Tail (per core, on its own batch row): med/mad via value-space bisection
  counting on fp16 residuals, the count pass split DVE (is_lt cache-reduce)
  || ACT (Sign-activation accumulate); both partials are combined by two
  accumulating PE matmuls. Bisquare weighted sum, row loss DMA'd out; host
  adds core0 + core4 rows.
"""

import sys

if '/opt/trn_rl_repo' not in sys.path:
    sys.path.insert(0, '/opt/trn_rl_repo')

import numpy as np

B, T, N_OBS, M_PRED = 2, 16, 4096, 2048
BT = B * T
NCORES = 8
F = BT // NCORES          # frames per core = 4
CH = N_OBS // 128         # obs chunks per frame = 32
COLS = F * CH             # d columns per core = 128
NROW = T * N_OBS          # residuals per batch row = 65536
K_MED = 32768.0           # rank (1-based) of lower median
TUNE = 4.6851
MADSTD = 0.67449

X1 = 440                  # PSUM-direct max cols per chunk (DVE tensor_reduce)
X2 = M_PRED - X1          # ACT-staged fp16 cols: DVE tensor_tensor max fold
                          # (2 read ports) then cache-reduce of the half
N_ITERS = 11              # bisection iterations (bracket [0,2*T0])
TAIL_ACT = False          # split tail count passes DVE || ACT (Sign trick)
FD1 = 280                 # tail count cols on DVE; rest Sign-counted on ACT
FD2 = 512 - FD1
GROUPS = [[0, 1, 2, 3], [4, 5, 6, 7]]

_CACHE = {}


def _build_nc(stage="D"):
    import concourse.bacc as bacc
    import concourse.tile as tile
    from concourse import mybir
    from contextlib import ExitStack

    A = mybir.AluOpType
    AF = mybir.ActivationFunctionType
    f32 = mybir.dt.float32
    f16 = mybir.dt.float16
    bf16 = mybir.dt.bfloat16
    X = mybir.AxisListType.X

    nc = bacc.Bacc("TRN2", target_bir_lowering=False, debug=False,
                   num_devices=NCORES)

    obs_in = nc.dram_tensor("obs_in", [13, F * N_OBS], bf16,
                            kind="ExternalInput").ap()
    pred_in = nc.dram_tensor("pred_in", [13, F * M_PRED], bf16,
                             kind="ExternalInput").ap()
    out_d = nc.dram_tensor("out", [1, 1], f32, kind="ExternalOutput").ap()

    def emit(tc, pp, stack):
        OBSL = pp.tile([13, F * N_OBS], bf16, name="OBSL", tag="OBSL")
        PREDL = pp.tile([13, F * M_PRED], bf16, name="PREDL", tag="PREDL")
        for f in range(F):
            nc.sync.dma_start(out=PREDL[:, f * M_PRED:(f + 1) * M_PRED],
                              in_=pred_in[:, f * M_PRED:(f + 1) * M_PRED])
            nc.sync.dma_start(out=OBSL[:, f * N_OBS:(f + 1) * N_OBS],
                              in_=obs_in[:, f * N_OBS:(f + 1) * N_OBS])

        zP = pp.tile([128, COLS], f32, name="zP", tag="zP")
        zG = pp.tile([128, COLS], f32, name="zG", tag="zG")
        junkG = pp.tile([128, X2], f16, name="junkG", tag="junkG")
        g = pp.tile([128, 512], f32, name="g", tag="g")

        dp = stack.enter_context(tc.tile_pool(name="dram", bufs=1,
                                              space="DRAM"))
        cc_in = []
        cc_out = []
        for h in range(2):
            cc_in.append(dp.tile([128, 64], f32, name=f"cc_in{h}"))
            cc_out.append(dp.tile([4, 128, 64], f32, name=f"cc_out{h}"))

        def gather_half(h):
            dh = pp.tile([128, 64], f32, name=f"dh{h}", tag=f"dh{h}")
            lo = h * 64
            nc.scalar.activation(out=dh, in_=zG[:, lo:lo + 64],
                                 func=AF.Relu, bias=0.0, scale=-2.0)
            nc.sync.dma_start(out=cc_in[h], in_=dh)
            nc.gpsimd.collective_compute(
                "AllGather", A.bypass, replica_groups=GROUPS,
                ins=[cc_in[h][:]], outs=[cc_out[h][:]])
            nc.sync.dma_start(
                out=g[:, h * 256:(h + 1) * 256].rearrange(
                    "p (r c) -> p r c", r=4),
                in_=cc_out[h].rearrange("r p c -> p r c"))

        # --- main loop: z' = -0.5*|a-b|^2 via K=13 bf16 matmul ------------
        with tc.tile_pool(name="mm", bufs=2, space="PSUM") as mmp, \
             tc.tile_pool(name="stg", bufs=4) as stgp:
            for f in range(F):
                for c in range(CH):
                    col = f * CH + c
                    ps = mmp.tile([128, M_PRED], f32, name="mmps", tag="mmps")
                    lhsT = OBSL[:, f * N_OBS + c * 128:
                                f * N_OBS + (c + 1) * 128]
                    for q in range(4):
                        nc.tensor.matmul(
                            ps[:, q * 512:(q + 1) * 512], lhsT=lhsT,
                            rhs=PREDL[:, f * M_PRED + q * 512:
                                      f * M_PRED + (q + 1) * 512],
                            start=True, stop=True)
                    staged = stgp.tile([128, X2], f16, name="stg", tag="stg")
                    nc.scalar.copy(out=staged, in_=ps[:, X1:M_PRED])
                    nc.vector.tensor_reduce(
                        out=zP[:, col:col + 1], in_=ps[:, 0:X1], axis=X,
                        op=A.max)
                    half = X2 // 2
                    quart = half // 2
                    fold = stgp.tile([128, half], f16, name="fold",
                                     tag="fold")
                    nc.vector.tensor_tensor(
                        out=fold, in0=staged[:, 0:half],
                        in1=staged[:, half:2 * half], op=A.max)
                    fold2 = stgp.tile([128, quart], f16, name="fold2",
                                      tag="fold2")
                    nc.vector.tensor_tensor(
                        out=fold2, in0=fold[:, 0:quart],
                        in1=fold[:, quart:2 * quart], op=A.max)
                    nc.vector.tensor_scalar(
                        out=junkG[:, 0:quart], in0=fold2, scalar1=-1e30,
                        scalar2=zP[:, col:col + 1], op0=A.max, op1=A.max,
                        accum_out=zG[:, col:col + 1])
                if f == 1:
                    gather_half(0)
            gather_half(1)

        # --- tail: med/mad via value bisection on fp16 residuals ----------
        r16 = pp.tile([128, 512], f16, name="r16", tag="r16")
        nc.scalar.activation(out=r16, in_=g, func=AF.Sqrt)

        ones128 = pp.tile([128, 128], f32, name="ones128", tag="ones128")
        nc.vector.memset(ones128, 1.0)
        halfm = pp.tile([128, 128], f32, name="halfm", tag="halfm")
        nc.vector.memset(halfm, 0.5)
        half1 = pp.tile([128, 1], f32, name="half1", tag="half1")
        nc.vector.memset(half1, 0.5)

        cnt = pp.tile([128, 1], f32, name="cnt", tag="cnt")
        acc = pp.tile([128, 1], f32, name="acc", tag="acc")
        dT = pp.tile([128, 1], f32, name="dT", tag="dT")
        jk16 = junkG[:, 0:FD1]
        jkA = pp.tile([128, FD2], f16, name="jkA", tag="jkA")

        bp = stack.enter_context(tc.tile_pool(name="bis_ps", bufs=2,
                                              space="PSUM"))

        # count(vals < T) split: DVE is_lt on cols [0:FD1], ACT Sign on
        # [FD1:512] (sum of sign(T - x) = c_below - c_above); combined by
        # two accumulating matmuls: tot = sum(cnt) + 0.5*sum(acc), compared
        # against K - 64*FD2.
        K_ADJ = K_MED - 64.0 * FD2

        def bisect(vals, tag, T0):
            Tt = pp.tile([128, 1], f32, name=f"T_{tag}", tag=f"T_{tag}")
            nc.vector.memset(Tt, T0)
            for j in range(N_ITERS):
                step = float(T0 / 2 ** (j + 1))
                tot = bp.tile([128, 1], f32, name=f"tot_{tag}", tag="tot")
                if TAIL_ACT:
                    nc.vector.tensor_scalar(
                        out=jk16, in0=vals[:, 0:FD1], scalar1=Tt[:, 0:1],
                        scalar2=None, op0=A.is_lt, op1=A.add, accum_out=cnt)
                    nc.scalar.activation(
                        out=jkA, in_=vals[:, FD1:512], func=AF.Sign,
                        bias=Tt[:, 0:1], scale=-1.0, accum_out=acc)
                    nc.tensor.matmul(tot, lhsT=ones128, rhs=cnt,
                                     start=True, stop=False)
                    nc.tensor.matmul(tot, lhsT=halfm, rhs=acc,
                                     start=False, stop=True)
                    kcmp = K_ADJ
                else:
                    nc.vector.tensor_scalar(
                        out=junkG[:, 0:512], in0=vals, scalar1=Tt[:, 0:1],
                        scalar2=None, op0=A.is_lt, op1=A.add, accum_out=cnt)
                    nc.tensor.matmul(tot, lhsT=ones128, rhs=cnt,
                                     start=True, stop=True)
                    kcmp = K_MED
                nc.vector.tensor_scalar(
                    out=dT, in0=tot, scalar1=kcmp, scalar2=2.0 * step,
                    op0=A.is_lt, op1=A.mult)
                nc.vector.scalar_tensor_tensor(
                    out=Tt, in0=dT, scalar=step, op0=A.subtract, op1=A.add,
                    in1=Tt)
            return Tt

        med = bisect(r16, "med", 2.0)
        negmed = pp.tile([128, 1], f32, name="negmed", tag="negmed")
        nc.vector.tensor_scalar(out=negmed, in0=med, scalar1=-1.0,
                                scalar2=None, op0=A.mult)
        u16 = pp.tile([128, 512], f16, name="u16", tag="u16")
        nc.scalar.activation(out=u16, in_=r16, func=AF.Abs,
                             bias=negmed[:, 0:1], scale=1.0)
        mad = bisect(u16, "mad", 0.5)

        # --- loss = 0.5 * sum(w * d), w = relu(1 - d/(TUNE*std)^2)^2 ------
        c1 = pp.tile([128, 1], f32, name="c1", tag="c1")
        nc.vector.tensor_scalar(out=c1, in0=mad, scalar1=TUNE / MADSTD,
                                scalar2=None, op0=A.mult)
        cs2 = pp.tile([128, 1], f32, name="cs2", tag="cs2")
        nc.vector.tensor_tensor(out=cs2, in0=c1, in1=c1, op=A.mult)
        inv = pp.tile([128, 1], f32, name="inv", tag="inv")
        nc.vector.reciprocal(inv, cs2)

        t1 = pp.tile([128, 512], f32, name="t1", tag="t1")
        nc.vector.tensor_scalar(out=t1, in0=g, scalar1=inv[:, 0:1],
                                scalar2=None, op0=A.mult)
        v = pp.tile([128, 512], f32, name="v", tag="v")
        nc.scalar.activation(out=v, in_=t1, func=AF.Relu,
                             bias=1.0, scale=-1.0)
        y = pp.tile([128, 512], f32, name="y", tag="y")
        nc.vector.tensor_tensor(out=y, in0=v, in1=g, op=A.mult)
        S = pp.tile([128, 1], f32, name="S", tag="S")
        jkf = pp.tile([128, 512], f32, name="jkf", tag="jkf")
        nc.vector.scalar_tensor_tensor(
            out=jkf, in0=y, scalar=1.0, op0=A.bypass, op1=A.mult,
            in1=v, accum_out=S)

        ls = bp.tile([1, 1], f32, name="ls")
        nc.tensor.matmul(ls, lhsT=half1, rhs=S, start=True, stop=True)
        ls_sb = pp.tile([1, 1], f32, name="ls_sb", tag="ls_sb")
        nc.scalar.copy(out=ls_sb, in_=ls)
        nc.sync.dma_start(out=out_d, in_=ls_sb)

    from contextlib import ExitStack
    with tile.TileContext(nc) as tc, ExitStack() as stack:
        pp = stack.enter_context(tc.tile_pool(name="persist", bufs=1))
        emit(tc, pp, stack)

    nc.compile()
    return nc


def _split16(x64, dt):
    hi = x64.astype(dt)
    lo = (x64 - hi.astype(np.float64)).astype(dt)
    return hi, lo


def _shard_inputs(points3d_obs, points3d_pred):
    import ml_dtypes
    bf16 = ml_dtypes.bfloat16
    obs = np.asarray(points3d_obs, dtype=np.float32).reshape(BT, N_OBS, 3)
    pred = np.asarray(points3d_pred, dtype=np.float32).reshape(BT, M_PRED, 3)
    in_maps = []
    for core in range(NCORES):
        so = obs[core * F:(core + 1) * F]       # [F, N, 3]
        sp = pred[core * F:(core + 1) * F]      # [F, M, 3]

        ha, la = _split16(so.astype(np.float64), bf16)
        hna, lna = _split16(-0.5 * (so.astype(np.float64) ** 2).sum(-1), bf16)
        hb, lb = _split16(sp.astype(np.float64), bf16)
        hnb, lnb = _split16(-0.5 * (sp.astype(np.float64) ** 2).sum(-1), bf16)

        onesN = np.ones((F, N_OBS), bf16)
        onesM = np.ones((F, M_PRED), bf16)

        # [13, F*N]: hi/lo(-0.5|a|^2), ha, la, ha, 1, 1
        obs_rows = np.stack([
            hna, lna,
            ha[..., 0], ha[..., 1], ha[..., 2],
            la[..., 0], la[..., 1], la[..., 2],
            ha[..., 0], ha[..., 1], ha[..., 2],
            onesN, onesN,
        ], axis=0).reshape(13, F * N_OBS)
        # [13, F*M]: 1, 1, hb, hb, lb, hi/lo(-0.5|b|^2)
        pred_rows = np.stack([
            onesM, onesM,
            hb[..., 0], hb[..., 1], hb[..., 2],
            hb[..., 0], hb[..., 1], hb[..., 2],
            lb[..., 0], lb[..., 1], lb[..., 2],
            hnb, lnb,
        ], axis=0).reshape(13, F * M_PRED)

        in_maps.append({
            "obs_in": np.ascontiguousarray(obs_rows),
            "pred_in": np.ascontiguousarray(pred_rows),
        })
    return in_maps


def _get_nc(stage="D"):
    key = f"nc_{stage}"
    if key not in _CACHE:
        _CACHE[key] = _build_nc(stage)
    return _CACHE[key]


def run(points3d_obs, points3d_pred, stage="D", **kwargs):
    """Run on hardware; kwargs forwarded to run_bass_kernel_spmd."""
    from concourse.bass_utils import run_bass_kernel_spmd
    nc = _get_nc(stage)
    in_maps = _shard_inputs(points3d_obs, points3d_pred)
    res = run_bass_kernel_spmd(nc, in_maps, list(range(NCORES)), **kwargs)
    return res


def kernel(points3d_obs, points3d_pred):
    res = run(points3d_obs, points3d_pred)
    loss = (np.float32(res.results[0]["out"][0, 0])
            + np.float32(res.results[4]["out"][0, 0]))
    return np.asarray(loss, dtype=np.float32).reshape(())
